# revision 10
# baseline (speedup 1.0000x reference)
"""CustomLSTM Trainium2 kernel (v2).

Problem: x [64, 1024, 256], LSTM(I=256, H=512), output h_last @ fc_w.T + fc_b -> [64, 1].

Strategy (data-parallel over batch, 8 cores x 8 sequences), all-bf16 matmuls:
- Gate pre-activations are computed TRANSPOSED: for each H-chunk j (128 dims)
  and gate, psum[j][:, gate*8+b] = sum_k V[k-chunk, (j,gate)-cols].T @ hT[k-chunk, b].
  V is the stationary operand (bf16, fast weight load), hT the moving operand
  (8 columns). This eliminates all per-step PE transposes: the activations and
  c/h updates run directly in [128 (h-dim), batch] layout, and the new hT is
  produced exactly in the layout the next step's matmuls consume.
- The 4 H-chunks accumulate in 4 separate PSUM banks so each chunk's
  activation chain starts as soon as its bank is complete, overlapping the
  remaining matmuls (k-outer matmul order delays h-chunk consumption).
- xproj = U.T @ x.T (+bias) is precomputed per 16-step chunk into SBUF
  ([128, t, (j,gate,b)] layout), bias folded in via ACT Identity+bias.
- Inputs (x, U, V, fc_w) are shipped as bf16 to halve axon transfer time.

Gate column order within a (j) block: [i, f, o, g], 8 batch cols each.
"""
import sys

if "/opt/trn_rl_repo" not in sys.path:
    sys.path.insert(0, "/opt/trn_rl_repo")

import numpy as np
import ml_dtypes
from contextlib import ExitStack

import concourse.bass as bass
import concourse.bacc as bacc
import concourse.tile as tile
import concourse.mybir as mybir
from concourse.bass_utils import run_bass_kernel_spmd

F32 = mybir.dt.float32
BF16 = mybir.dt.bfloat16
F8 = mybir.dt.float8e4
AF = mybir.ActivationFunctionType
ALU = mybir.AluOpType
BF16_NP = ml_dtypes.bfloat16
F8_NP = ml_dtypes.float8_e4m3

# fp8 recurrence (V e4m3 x16, h e4m3, 1/16 descale in the gates add) was
# tried and REJECTED: HW fp8 is much lossier than the ml_dtypes model
# (rel err 1.9e-2 at 32 steps vs 2e-3 simulated; gate is 2e-2 at 1024).
# bf16 measures 2.8e-3 end-to-end. Keep False.
V_FP8 = False
VSCALE = 16.0

B, T, I, H = 64, 1024, 256, 512
NCORES = 8
BC = B // NCORES           # 8 sequences per core
G4 = 4 * H                 # 2048
TC = 16                    # timesteps per xproj chunk
NJ = 4                     # H chunks of 128
NK = 4                     # contraction chunks of 128 for V
NKI = 2                    # contraction chunks for U


def build_program(n_steps=T):
    nc = bacc.Bacc("TRN2", target_bir_lowering=False, debug=False,
                   num_devices=NCORES)

    xc = nc.dram_tensor("xc", [BC, T, I], BF16, kind="ExternalInput")
    Vc = nc.dram_tensor("Vc", [H, G4], F8 if V_FP8 else BF16, kind="ExternalInput")
    Uc = nc.dram_tensor("Uc", [I, G4], BF16, kind="ExternalInput")
    bT = nc.dram_tensor("bT", [128, 16], F32, kind="ExternalInput")
    eye = nc.dram_tensor("eye", [128, 128], BF16, kind="ExternalInput")
    fcw = nc.dram_tensor("fcw", [128, NJ], BF16, kind="ExternalInput")
    fcb = nc.dram_tensor("fcb", [1, BC], F32, kind="ExternalInput")
    y8 = nc.dram_tensor("y8", [1, BC], F32, kind="ExternalOutput")

    n_chunks = (n_steps + TC - 1) // TC

    with ExitStack() as ctx:
        tc_ = ctx.enter_context(tile.TileContext(nc))

        consts = ctx.enter_context(tc_.tile_pool(name="consts", bufs=1))
        xpool = ctx.enter_context(tc_.tile_pool(name="xpool", bufs=2))
        xppool = ctx.enter_context(tc_.tile_pool(name="xppool", bufs=3))
        work = ctx.enter_context(tc_.tile_pool(name="work", bufs=2))
        state = ctx.enter_context(tc_.tile_pool(name="state", bufs=2))

        ps_step = ctx.enter_context(tc_.tile_pool(name="ps_s", bufs=1, space="PSUM"))
        ps_x = ctx.enter_context(tc_.tile_pool(name="ps_x", bufs=2, space="PSUM"))
        ps_t = ctx.enter_context(tc_.tile_pool(name="ps_t", bufs=2, space="PSUM"))

        # ---- constants: DMA to staging, DVE relay so consumers have 1-sem deps
        def relay(dram_ap, shape, dtype, tag):
            st = consts.tile(shape, dtype, tag=f"{tag}_st")
            nc.sync.dma_start(st[:], dram_ap)
            dst = consts.tile(shape, dtype, tag=tag)
            nc.vector.tensor_copy(dst[:], st[:])
            return dst

        V_sb = [relay(Vc[k * 128:(k + 1) * 128, :], [128, G4], F8 if V_FP8 else BF16, f"V{k}")
                for k in range(NK)]
        U_sb = [relay(Uc[k * 128:(k + 1) * 128, :], [128, G4], BF16, f"U{k}")
                for k in range(NKI)]
        bT_sb = relay(bT[:], [128, 16], F32, "bT")
        eye_sb = relay(eye[:], [128, 128], BF16, "eye")
        fcw_sb = relay(fcw[:], [128, NJ], BF16, "fcw")
        fcb_sb = relay(fcb[:], [1, BC], F32, "fcb")

        # ---- initial state h=0, c=0
        c_t = state.tile([128, NJ * BC], F32, tag="c")
        nc.vector.memset(c_t[:], 0.0)
        hT = state.tile([128, NJ * BC], F8 if V_FP8 else BF16, tag="h")
        nc.vector.memset(hT[:], 0.0)

        xproj_chunks = [None] * n_chunks

        def emit_xproj_chunk(ci):
            """xprojT for steps [ci*TC, (ci+1)*TC) -> sbuf [128, TC, 128] f32.

            partition = h-dim within chunk; cols = (t, j*32 + gate*8 + b)."""
            x_t = xpool.tile([128, I], BF16, tag="x")
            nc.sync.dma_start(
                x_t[:],
                xc[:, ci * TC:(ci + 1) * TC, :].rearrange("b t i -> t b i"))
            xT = []
            for k in range(NKI):
                pT = ps_t.tile([128, 128], BF16, tag="pst")
                nc.tensor.transpose(pT[:], x_t[:, k * 128:(k + 1) * 128],
                                    eye_sb[:])
                xk = xpool.tile([128, 128], BF16, tag=f"xT{k}")
                nc.vector.tensor_copy(xk[:], pT[:])
                xT.append(xk)
            xp = xppool.tile([128, TC, 128], F32, tag="xp")
            for jg in range(16):
                pX = ps_x.tile([128, TC, BC], F32, tag="psx")
                nc.tensor.matmul(pX[:], U_sb[0][:, jg * 128:(jg + 1) * 128],
                                 xT[0][:], start=True, stop=False)
                nc.tensor.matmul(pX[:], U_sb[1][:, jg * 128:(jg + 1) * 128],
                                 xT[1][:], start=False, stop=True)
                j, gate = jg // 4, jg % 4
                c0 = j * 32 + gate * BC
                nc.scalar.activation(xp[:, :, c0:c0 + BC], pX[:], AF.Identity,
                                     bias=bT_sb[:, jg:jg + 1])
            xproj_chunks[ci] = xp

        emit_xproj_chunk(0)
        if n_chunks > 1:
            emit_xproj_chunk(1)

        for t in range(n_steps):
            ci, tl = divmod(t, TC)
            xp = xproj_chunks[ci]
            psj = [ps_step.tile([128, 4 * BC], F32, tag=f"g{j}", name=f"g{j}")
                   for j in range(NJ)]
            # Hybrid order: the k=0 sweep first (only needs h-chunk 0, so it
            # starts while the previous step's later h-chunks finish), then
            # per-bank k=1..3 blocks so each PSUM bank completes early and
            # its add/activation chain overlaps the remaining matmuls.
            def emit_mm(k, j, gate):
                jg = j * 4 + gate
                nc.tensor.matmul(
                    psj[j][:, gate * BC:(gate + 1) * BC],
                    V_sb[k][:, jg * 128:(jg + 1) * 128],
                    hT[:, k * BC:(k + 1) * BC],
                    start=(k == 0 and gate == 0),
                    stop=(k == NK - 1 and gate == 3),
                    skip_group_check=True)

            for j in range(NJ):
                for gate in range(4):
                    emit_mm(0, j, gate)
            for j in range(NJ):
                for k in range(1, NK):
                    for gate in range(4):
                        emit_mm(k, j, gate)

            new_h = state.tile([128, NJ * BC], F8 if V_FP8 else BF16, tag="h")
            new_c = state.tile([128, NJ * BC], F32, tag="c")
            # gts cols: (j, gate, b) — per-j adds (each gated only on its own
            # PSUM bank), then gate-wise ops batched across j via strided APs.
            gts = work.tile([128, NJ, 4, BC], F32, tag="gt")
            for j in range(NJ):
                if V_FP8:
                    nc.vector.scalar_tensor_tensor(
                        gts[:, j:j + 1, :, :].squeeze(1), psj[j][:],
                        1.0 / VSCALE,
                        xp[:, tl:tl + 1, j * 32:(j + 1) * 32].squeeze(1),
                        ALU.mult, ALU.add)
                else:
                    nc.vector.tensor_add(
                        gts[:, j:j + 1, :, :].squeeze(1), psj[j][:],
                        xp[:, tl:tl + 1, j * 32:(j + 1) * 32].squeeze(1))
            acts = work.tile([128, NJ, 3, BC], F32, tag="ac")
            nc.scalar.activation(acts[:], gts[:, :, 0:3, :], AF.Sigmoid)
            gg = work.tile([128, NJ, BC], F32, tag="gg")
            nc.scalar.activation(gg[:], gts[:, :, 3, :], AF.Tanh)
            ig = work.tile([128, NJ, BC], F32, tag="ig")
            nc.vector.tensor_mul(ig[:], acts[:, :, 0, :], gg[:])
            fcx = work.tile([128, NJ, BC], F32, tag="fc")
            nc.vector.tensor_mul(fcx[:], acts[:, :, 1, :], c_t[:])
            # Everything below sig/tanh is gated on all four bank-adds anyway,
            # so batch c / tanh(c) / h into single ops (DVE serializes
            # same-engine ops; one [128,32] op finishes sooner than four
            # [128,8] ones and unblocks next-step matmuls earlier).
            nc.vector.tensor_add(new_c[:], ig[:], fcx[:])
            tca = work.tile([128, NJ * BC], F32, tag="tca")
            nc.scalar.activation(tca[:], new_c[:], AF.Tanh)
            nc.vector.tensor_mul(new_h[:], acts[:, :, 2, :], tca[:])
            hT, c_t = new_h, new_c

            if tl == 6 and ci + 2 < n_chunks:
                emit_xproj_chunk(ci + 2)

        # ---- final FC: y = fcw.T-reduced @ hT + fcb
        if V_FP8:
            hT_bf = consts.tile([128, NJ * BC], BF16, tag="hbf")
            nc.vector.tensor_copy(hT_bf[:], hT[:])
            hT = hT_bf
        pf = ps_x.tile([1, BC], F32, tag="psx")
        for j in range(NJ):
            nc.tensor.matmul(pf[:], fcw_sb[:, j:j + 1],
                             hT[:, j * BC:(j + 1) * BC],
                             start=(j == 0), stop=(j == NJ - 1))
        y_sb = consts.tile([1, BC], F32, tag="y")
        nc.vector.tensor_add(y_sb[:], pf[:], fcb_sb[:])
        nc.sync.dma_start(y8[:], y_sb[:])

    nc.compile()
    return nc


def prep_inputs(x, U_i, V_i, b_i, U_f, V_f, b_f, U_h, V_h, b_h, U_o, V_o, b_o,
                fc_w, fc_b):
    # column order: for j in 0..3 (H-chunk), gates [i, f, o, g], 128 cols each
    def perm_cols(Ws, scale=1.0, dt=BF16_NP):
        cols = []
        for j in range(NJ):
            for W in Ws:
                cols.append(
                    np.asarray(W, np.float32)[:, j * 128:(j + 1) * 128] * scale)
        return np.ascontiguousarray(np.concatenate(cols, axis=1).astype(dt))

    if V_FP8:
        V_cat = perm_cols([V_i, V_f, V_o, V_h], scale=VSCALE, dt=F8_NP)
    else:
        V_cat = perm_cols([V_i, V_f, V_o, V_h])
    U_cat = perm_cols([U_i, U_f, U_o, U_h])
    bTn = np.zeros((128, 16), np.float32)
    for j in range(NJ):
        for gi, bv in enumerate([b_i, b_f, b_o, b_h]):
            bTn[:, j * 4 + gi] = np.asarray(bv, np.float32)[j * 128:(j + 1) * 128]
    fcwT = np.ascontiguousarray(
        np.asarray(fc_w, np.float32).reshape(NJ, 128).T.astype(BF16_NP))
    fcbn = np.full((1, BC), float(np.asarray(fc_b).reshape(-1)[0]), np.float32)
    shared = {
        "Vc": V_cat, "Uc": U_cat, "bT": bTn,
        "eye": np.eye(128, dtype=BF16_NP),
        "fcw": fcwT, "fcb": fcbn,
    }
    x = np.asarray(x, np.float32).astype(BF16_NP)
    in_maps = []
    for c in range(NCORES):
        m = dict(shared)
        m["xc"] = np.ascontiguousarray(x[c * BC:(c + 1) * BC])
        in_maps.append(m)
    return in_maps


_CACHED = {}


def kernel(**inputs) -> np.ndarray:
    in_maps = prep_inputs(**inputs)
    if "nc" not in _CACHED:
        _CACHED["nc"] = build_program()
    nc = _CACHED["nc"]
    res = run_bass_kernel_spmd(nc, in_maps, core_ids=list(range(NCORES)))
    _CACHED["last_results"] = res
    out = np.empty((B, 1), np.float32)
    for c in range(NCORES):
        out[c * BC:(c + 1) * BC, 0] = res.results[c]["y8"][0]
    return out


if __name__ == "__main__":
    import reference

    inputs = {k: np.asarray(v) for k, v in reference.setup_inputs().items()}
    exp = np.asarray(reference.reference(**inputs))
    got = kernel(**inputs)
    err = np.abs(got - exp).max()
    rel = np.linalg.norm(got - exp) / np.linalg.norm(exp)
    print(f"max abs err: {err:.3e}  rel err: {rel:.3e}")
